# revision 42
# baseline (speedup 1.0000x reference)
"""Trainium2 Bass kernel for nn_DS4DKernel_56504589746318.

Math (per batch b):
    deltaA = W @ du[b]              # (N=64, L=4096)
    S      = cumsum_L(deltaA)       # (64, 4096)  -- tensor_tensor_scan
    K[b]   = (C*Bvec) @ S + base    # (H=1024, L=4096), base = C @ (A @ Bvec)

Sharding: data-parallel over batch, one batch per NeuronCore (B=8 = 8 cores).
Small matrices (W^T, (C*Bvec)^T, base) are precomputed on host and replicated.

The kernel is HBM-bound, so du is sent and K returned as bf16 (rel-err
~5e-3, well inside the 2e-2 gate), halving traffic to ~16.3 MiB/core.
The host pre-permutes du into the SBUF tile layout so every load is a
long contiguous run per partition, and un-permutes the output.

Schedule notes:
 - input du on the qSP HWDGE ring (sync engine), half-tile loads queued
   up front (first two tiles in halves for an earlier mm1 start);
   mm1/scan run at 512-col half-tile grain chasing each landing load.
 - a short junk-matmul burst runs during the preamble to open the HAM
   clock-gate (PE defaults to 1.2 GHz; sustained activity raises it to
   2.4 GHz ~10us later) before real matmuls begin.
 - mm2 + PSUM evacuation at 1024-col grain; each chunk's copy is split
   between DVE and ACT (both stuck at 1x for f32 PSUM sources) so the
   PSUM buffer frees in ~0.7us, and psB holds 3 bufs of cushion.  The
   last mm1 blocks are interleaved into the mm2(2) chunk stream to keep
   PE duty up through the copy-bound drain.
 - output stores: early l-tiles on qAct; drain l-tiles split across
   both HWDGE rings (qSP is empty once input is done).
"""

import sys

for _p in ("/opt/trn_rl_repo", "/root/.axon_site/_ro/trn_rl_repo"):
    if _p not in sys.path:
        sys.path.insert(0, _p)

import numpy as np
import ml_dtypes

import concourse.bass as bass
import concourse.mybir as mybir
import concourse.tile as tile
from concourse import bacc
from concourse.bass_utils import run_bass_kernel_spmd

B, H, N, L = 8, 1024, 64, 4096
P = 128          # SBUF partitions
HC = H // P      # 8 h-chunks of 128
LT = 1024        # output l-tile width
NLT = L // LT    # 4 output l-tiles
LH = 512         # input half-tile width (matmul moving dim, 1 PSUM bank)
NH = L // LH     # 8 half-tiles

F32 = mybir.dt.float32
BF16 = mybir.dt.bfloat16
ADD = mybir.AluOpType.add
BYPASS = mybir.AluOpType.bypass

BF16_NP = np.dtype(ml_dtypes.bfloat16)


def build_nc():
    nc = bacc.Bacc()
    # du in DRAM pre-permuted: [p, ht, c, j] = du[h=c*128+p, l=ht*LH+j]
    # out in DRAM: [p, lt, c, j] = K[h=c*128+p, l=lt*LT+j]
    du_d = nc.declare_dram_parameter("du", [P, NH, HC, LH], BF16, isOutput=False)
    wt_d = nc.declare_dram_parameter("wt", [P, HC, N], BF16, isOutput=False)
    ccbt_d = nc.declare_dram_parameter("ccbt", [N, H], BF16, isOutput=False)
    base_d = nc.declare_dram_parameter("base", [P, HC], F32, isOutput=False)
    out_d = nc.declare_dram_parameter("out", [P, NLT, HC, LT], BF16, isOutput=True)

    with tile.TileContext(nc) as tc:
        with (
            tc.tile_pool(name="const", bufs=1) as cpool,
            tc.tile_pool(name="du", bufs=8) as dupool,
            tc.tile_pool(name="s", bufs=6) as spool,
            tc.tile_pool(name="outp", bufs=2) as opool,
            tc.tile_pool(name="psA", bufs=2, space="PSUM") as psA,
            tc.tile_pool(name="psB", bufs=3, space="PSUM") as psB,
        ):
            du_t = [None] * NH
            dA_t = [None] * NH
            S_t = [None] * NH

            def load_du(ht, split=1):
                du_t[ht] = dupool.tile([P, HC, LH], BF16, tag="du_t", name="du_t")
                for g in range(split):
                    c0, c1 = g * HC // split, (g + 1) * HC // split
                    nc.sync.dma_start(
                        du_t[ht][:, c0:c1, :], du_d[:, ht, c0:c1, :]
                    )

            # HAM warm-up source: a small bf16 zero tile, memset FIRST on
            # the vector engine so the junk matmuls can start as soon as
            # the engines come up (~5.5us), well before the first du
            # half-tile lands.
            bzero_sb = cpool.tile([N, LH], BF16)
            nc.vector.memset(bzero_sb[:], 0.0)

            # qSP ring order: du0 first half, wt (so mm1 can start on the
            # first half), du0 second half, other consts, du 1..7.
            du_t[0] = dupool.tile([P, HC, LH], BF16, tag="du_t", name="du_t")
            nc.sync.dma_start(du_t[0][:, 0 : HC // 2, :], du_d[:, 0, 0 : HC // 2, :])
            wt_sb = cpool.tile([P, HC, N], BF16)     # [p, c, n] = W^T[c*128+p, n]
            nc.sync.dma_start(wt_sb[:], wt_d[:, :, :])
            nc.sync.dma_start(du_t[0][:, HC // 2 :, :], du_d[:, 0, HC // 2 :, :])

            ccbt_sb = cpool.tile([N, H], BF16)       # [n, h] = (C*Bvec)^T
            nc.sync.dma_start(ccbt_sb[:], ccbt_d[:, :])
            base_sb = cpool.tile([P, HC], F32)       # [p, c] = base[c*128+p]
            nc.sync.dma_start(base_sb[:], base_d[:, :])
            zeros_sb = cpool.tile([N, LH], F32)      # data1 for the scan
            nc.vector.memset(zeros_sb[:], 0.0)

            load_du(1, split=2)
            for ht in range(2, NH):
                load_du(ht)

            # warm-up burst: ~4 cold matmuls ~= 1.7us of PE activity,
            # ending about when the first du half-load lands
            for i in range(4):
                junk_po = psA.tile([N, LH], F32, tag="dA_t", name="warmup")
                nc.tensor.matmul(
                    junk_po[:, :], bzero_sb[:, 0:N], bzero_sb[:, :],
                    start=True, stop=True,
                )

            def mm1(ht):
                # deltaA half-tile: accumulate over 8 h-chunks into PSUM
                dA_t[ht] = psA.tile([N, LH], F32, tag="dA_t", name="dA_t")
                for c in range(HC):
                    nc.tensor.matmul(
                        dA_t[ht][:, :],
                        wt_sb[:, c, :],
                        du_t[ht][:, c, :],
                        start=(c == 0),
                        stop=(c == HC - 1),
                    )

            def scan(ht):
                S_t[ht] = spool.tile([N, LH], BF16, tag="S_t", name="S_t")
                initial = 0.0 if ht == 0 else S_t[ht - 1][:, LH - 1 : LH]
                nc.vector.tensor_tensor_scan(
                    S_t[ht][:], dA_t[ht][:], zeros_sb[:], initial,
                    op0=ADD, op1=BYPASS,
                )

            def mm2_range(lt, out_sb, c_lo, c_hi, store_plan, g0):
                # store_plan: list of (chunk_excl_end, engine) store groups
                gi = 0
                for c in range(c_lo, c_hi):
                    po = psB.tile([P, LT], F32, tag="po", name="po")
                    for s in range(2):
                        nc.tensor.matmul(
                            po[:, s * LH : (s + 1) * LH],
                            ccbt_sb[:, c * P : (c + 1) * P],
                            S_t[2 * lt + s][:, :],
                            start=True,
                            stop=True,
                        )
                    # PSUM -> SBUF with fused "+ base[h]" and f32->bf16
                    # downcast.  Each chunk is split between DVE and ACT so
                    # the PSUM buffer frees in ~0.7us instead of ~1.3us --
                    # psB turnaround is what gates the mm2 chunk stream.
                    for s, eng in ((0, nc.vector.tensor_scalar_add), (1, nc.scalar.add)):
                        eng(
                            out_sb[:, c, s * LH : (s + 1) * LH],
                            po[:, s * LH : (s + 1) * LH],
                            base_sb[:, c : c + 1],
                        )
                    if gi < len(store_plan) and c + 1 == store_plan[gi][0]:
                        eng = store_plan[gi][1]
                        eng.dma_start(
                            out_d[:, lt, g0 : c + 1, :], out_sb[:, g0 : c + 1, :]
                        )
                        g0 = c + 1
                        gi += 1

            def mm2_and_out(lt, store_plan):
                out_sb = opool.tile([P, HC, LT], BF16, tag="out_sb", name="out_sb")
                mm2_range(lt, out_sb, 0, HC, store_plan, 0)

            # PE program order: mm1(0) mm1(1) mm2(0) mm1(2) mm1(3) mm2(1)
            # mm1(4) mm1(5) [mm2(2) c0-3] mm1(6) [mm2(2) c4-7] mm1(7)
            # mm2(3).  mm2 runs as early as its scans allow so the PE has
            # ready work while loads land, and the dense mm1(6)/mm1(7)
            # blocks are interleaved into the copy-bound mm2(2) chunk
            # stream to keep PE duty (and the HAM clock) up in the drain.
            A, S_, G = nc.scalar, nc.sync, nc.gpsimd
            for lt in range(2):
                mm1(2 * lt)
                scan(2 * lt)
                mm1(2 * lt + 1)
                scan(2 * lt + 1)
                mm2_and_out(lt, [(4, G), (8, G)])
            mm1(4)
            scan(4)
            mm1(5)
            scan(5)
            osb2 = opool.tile([P, HC, LT], BF16, tag="out_sb", name="osb2")
            mm2_range(2, osb2, 0, 4, [(4, S_)], 0)
            mm1(6)
            scan(6)
            mm2_range(2, osb2, 4, HC, [(8, S_)], 4)
            mm1(7)
            scan(7)
            mm2_and_out(3, [(2, S_), (4, S_), (6, S_), (8, S_)])

    nc.compile()
    return nc


_NC_CACHE = None


def _get_nc():
    global _NC_CACHE
    if _NC_CACHE is None:
        _NC_CACHE = build_nc()
    return _NC_CACHE


def _prep_in_maps(du, C, Bvec, A, W):
    du = np.asarray(du, dtype=np.float32)
    C = np.asarray(C, dtype=np.float32)
    Bvec = np.asarray(Bvec, dtype=np.float32)
    A = np.asarray(A, dtype=np.float32)
    W = np.asarray(W, dtype=np.float32)

    # [b, p, ht, c, j] bf16, contiguous per core
    du_pre = np.ascontiguousarray(
        du.reshape(B, HC, P, NH, LH).transpose(0, 2, 3, 1, 4).astype(BF16_NP)
    )
    wt = W.T.reshape(HC, P, N).transpose(1, 0, 2).astype(BF16_NP)  # (P, HC, N)
    wt = np.ascontiguousarray(wt)
    ccbt = np.ascontiguousarray((C * Bvec[None, :]).T.astype(BF16_NP))  # (N, H)
    base = C @ (A @ Bvec)                                 # (H,)
    base_t = np.ascontiguousarray(base.reshape(HC, P).T)  # (P, HC)

    return [
        {"du": du_pre[b], "wt": wt, "ccbt": ccbt, "base": base_t}
        for b in range(B)
    ]


def run(du, C, Bvec, A, W, trace=False):
    nc = _get_nc()
    in_maps = _prep_in_maps(du, C, Bvec, A, W)
    res = run_bass_kernel_spmd(nc, in_maps, core_ids=list(range(B)), trace=trace)
    out_pre = np.stack([res.results[b]["out"] for b in range(B)], axis=0)
    # [b, p, lt, c, j] -> [b, c, p, lt, j] -> (B, H, L) f32
    out = (
        out_pre.transpose(0, 3, 1, 2, 4).astype(np.float32).reshape(B, H, L)
    )
    return out, res


def kernel(du, C, Bvec, A, W):
    out, _ = run(du, C, Bvec, A, W, trace=False)
    return out


# revision 43
# speedup vs baseline: 1.0136x; 1.0136x over previous
"""Trainium2 Bass kernel for nn_DS4DKernel_56504589746318.

Math (per batch b):
    deltaA = W @ du[b]              # (N=64, L=4096)
    S      = cumsum_L(deltaA)       # (64, 4096)  -- tensor_tensor_scan
    K[b]   = (C*Bvec) @ S + base    # (H=1024, L=4096), base = C @ (A @ Bvec)

Sharding: data-parallel over batch, one batch per NeuronCore (B=8 = 8 cores).
Small matrices (W^T, (C*Bvec)^T, base) are precomputed on host and replicated.

The kernel is HBM-bound, so du is sent and K returned as bf16 (rel-err
~5e-3, well inside the 2e-2 gate), halving traffic to ~16.3 MiB/core.
The host pre-permutes du into the SBUF tile layout so every load is a
long contiguous run per partition, and un-permutes the output.

Schedule notes:
 - input du on the qSP HWDGE ring (sync engine), half-tile loads queued
   up front (first two tiles in halves for an earlier mm1 start);
   mm1/scan run at 512-col half-tile grain chasing each landing load.
 - a short junk-matmul burst runs during the preamble to open the HAM
   clock-gate (PE defaults to 1.2 GHz; sustained activity raises it to
   2.4 GHz ~10us later) before real matmuls begin.
 - mm2 + PSUM evacuation at 1024-col grain; each chunk's copy is split
   between DVE and ACT (both stuck at 1x for f32 PSUM sources) so the
   PSUM buffer frees in ~0.7us, and psB holds 3 bufs of cushion.  The
   last mm1 blocks are interleaved into the mm2(2) chunk stream to keep
   PE duty up through the copy-bound drain.
 - output stores: early l-tiles on qAct; drain l-tiles split across
   both HWDGE rings (qSP is empty once input is done).
"""

import sys

for _p in ("/opt/trn_rl_repo", "/root/.axon_site/_ro/trn_rl_repo"):
    if _p not in sys.path:
        sys.path.insert(0, _p)

import numpy as np
import ml_dtypes

import concourse.bass as bass
import concourse.mybir as mybir
import concourse.tile as tile
from concourse import bacc
from concourse.bass_utils import run_bass_kernel_spmd

B, H, N, L = 8, 1024, 64, 4096
P = 128          # SBUF partitions
HC = H // P      # 8 h-chunks of 128
LT = 1024        # output l-tile width
NLT = L // LT    # 4 output l-tiles
LH = 512         # input half-tile width (matmul moving dim, 1 PSUM bank)
NH = L // LH     # 8 half-tiles

F32 = mybir.dt.float32
BF16 = mybir.dt.bfloat16
ADD = mybir.AluOpType.add
BYPASS = mybir.AluOpType.bypass

BF16_NP = np.dtype(ml_dtypes.bfloat16)


def build_nc():
    nc = bacc.Bacc()
    # du in DRAM pre-permuted: [p, ht, c, j] = du[h=c*128+p, l=ht*LH+j]
    # out in DRAM: [p, lt, c, j] = K[h=c*128+p, l=lt*LT+j]
    du_d = nc.declare_dram_parameter("du", [P, NH, HC, LH], BF16, isOutput=False)
    wt_d = nc.declare_dram_parameter("wt", [P, HC, N], BF16, isOutput=False)
    ccbt_d = nc.declare_dram_parameter("ccbt", [N, H], BF16, isOutput=False)
    base_d = nc.declare_dram_parameter("base", [P, HC], F32, isOutput=False)
    out_d = nc.declare_dram_parameter("out", [P, NLT, HC, LT], BF16, isOutput=True)

    with tile.TileContext(nc) as tc:
        with (
            tc.tile_pool(name="const", bufs=1) as cpool,
            tc.tile_pool(name="du", bufs=8) as dupool,
            tc.tile_pool(name="s", bufs=6) as spool,
            tc.tile_pool(name="outp", bufs=2) as opool,
            tc.tile_pool(name="psA", bufs=2, space="PSUM") as psA,
            tc.tile_pool(name="psB", bufs=3, space="PSUM") as psB,
        ):
            du_t = [None] * NH
            dA_t = [None] * NH
            S_t = [None] * NH

            def load_du(ht, split=1):
                du_t[ht] = dupool.tile([P, HC, LH], BF16, tag="du_t", name="du_t")
                for g in range(split):
                    c0, c1 = g * HC // split, (g + 1) * HC // split
                    nc.sync.dma_start(
                        du_t[ht][:, c0:c1, :], du_d[:, ht, c0:c1, :]
                    )

            # HAM warm-up source: a small bf16 zero tile, memset FIRST on
            # the vector engine so the junk matmuls can start as soon as
            # the engines come up (~5.5us), well before the first du
            # half-tile lands.
            bzero_sb = cpool.tile([N, LH], BF16)
            nc.vector.memset(bzero_sb[:], 0.0)

            # qSP ring order: du0 first half, wt (so mm1 can start on the
            # first half), du0 second half, other consts, du 1..7.
            du_t[0] = dupool.tile([P, HC, LH], BF16, tag="du_t", name="du_t")
            nc.sync.dma_start(du_t[0][:, 0 : HC // 2, :], du_d[:, 0, 0 : HC // 2, :])
            wt_sb = cpool.tile([P, HC, N], BF16)     # [p, c, n] = W^T[c*128+p, n]
            nc.sync.dma_start(wt_sb[:], wt_d[:, :, :])
            nc.sync.dma_start(du_t[0][:, HC // 2 :, :], du_d[:, 0, HC // 2 :, :])

            ccbt_sb = cpool.tile([N, H], BF16)       # [n, h] = (C*Bvec)^T
            nc.sync.dma_start(ccbt_sb[:], ccbt_d[:, :])
            base_sb = cpool.tile([P, HC], F32)       # [p, c] = base[c*128+p]
            nc.sync.dma_start(base_sb[:], base_d[:, :])
            zeros_sb = cpool.tile([N, LH], F32)      # data1 for the scan
            nc.vector.memset(zeros_sb[:], 0.0)

            load_du(1, split=2)
            for ht in range(2, NH):
                load_du(ht)

            # warm-up burst: ~4 cold matmuls ~= 1.7us of PE activity,
            # ending about when the first du half-load lands
            for i in range(4):
                junk_po = psA.tile([N, LH], F32, tag="dA_t", name="warmup")
                nc.tensor.matmul(
                    junk_po[:, :], bzero_sb[:, 0:N], bzero_sb[:, :],
                    start=True, stop=True,
                )

            def mm1(ht):
                # deltaA half-tile: accumulate over 8 h-chunks into PSUM
                dA_t[ht] = psA.tile([N, LH], F32, tag="dA_t", name="dA_t")
                for c in range(HC):
                    nc.tensor.matmul(
                        dA_t[ht][:, :],
                        wt_sb[:, c, :],
                        du_t[ht][:, c, :],
                        start=(c == 0),
                        stop=(c == HC - 1),
                    )

            def scan(ht):
                S_t[ht] = spool.tile([N, LH], BF16, tag="S_t", name="S_t")
                initial = 0.0 if ht == 0 else S_t[ht - 1][:, LH - 1 : LH]
                nc.vector.tensor_tensor_scan(
                    S_t[ht][:], dA_t[ht][:], zeros_sb[:], initial,
                    op0=ADD, op1=BYPASS,
                )

            def mm2_range(lt, out_sb, c_lo, c_hi, store_plan, g0):
                # store_plan: list of (chunk_excl_end, engine) store groups
                gi = 0
                for c in range(c_lo, c_hi):
                    po = psB.tile([P, LT], F32, tag="po", name="po")
                    for s in range(2):
                        nc.tensor.matmul(
                            po[:, s * LH : (s + 1) * LH],
                            ccbt_sb[:, c * P : (c + 1) * P],
                            S_t[2 * lt + s][:, :],
                            start=True,
                            stop=True,
                        )
                    # PSUM -> SBUF with fused "+ base[h]" and f32->bf16
                    # downcast.  Each chunk is split between DVE and ACT so
                    # the PSUM buffer frees in ~0.7us instead of ~1.3us --
                    # psB turnaround is what gates the mm2 chunk stream.
                    for s, eng in ((0, nc.vector.tensor_scalar_add), (1, nc.scalar.add)):
                        eng(
                            out_sb[:, c, s * LH : (s + 1) * LH],
                            po[:, s * LH : (s + 1) * LH],
                            base_sb[:, c : c + 1],
                        )
                    if gi < len(store_plan) and c + 1 == store_plan[gi][0]:
                        eng = store_plan[gi][1]
                        eng.dma_start(
                            out_d[:, lt, g0 : c + 1, :], out_sb[:, g0 : c + 1, :]
                        )
                        g0 = c + 1
                        gi += 1

            def mm2_and_out(lt, store_plan):
                out_sb = opool.tile([P, HC, LT], BF16, tag="out_sb", name="out_sb")
                mm2_range(lt, out_sb, 0, HC, store_plan, 0)

            # PE program order: mm1(0) mm1(1) mm2(0) mm1(2) mm1(3) mm2(1)
            # mm1(4) mm1(5) [mm2(2) c0-3] mm1(6) [mm2(2) c4-7] mm1(7)
            # mm2(3).  mm2 runs as early as its scans allow so the PE has
            # ready work while loads land, and the dense mm1(6)/mm1(7)
            # blocks are interleaved into the copy-bound mm2(2) chunk
            # stream to keep PE duty (and the HAM clock) up in the drain.
            A, S_ = nc.scalar, nc.sync
            for lt in range(2):
                mm1(2 * lt)
                scan(2 * lt)
                mm1(2 * lt + 1)
                scan(2 * lt + 1)
                mm2_and_out(lt, [(4, A), (8, A)])
            mm1(4)
            scan(4)
            mm1(5)
            scan(5)
            osb2 = opool.tile([P, HC, LT], BF16, tag="out_sb", name="osb2")
            mm2_range(2, osb2, 0, 4, [(4, S_)], 0)
            mm1(6)
            scan(6)
            mm2_range(2, osb2, 4, HC, [(8, S_)], 4)
            mm1(7)
            scan(7)
            mm2_and_out(3, [(2, S_), (4, S_), (6, S_), (8, S_)])

    nc.compile()
    return nc


_NC_CACHE = None


def _get_nc():
    global _NC_CACHE
    if _NC_CACHE is None:
        _NC_CACHE = build_nc()
    return _NC_CACHE


def _prep_in_maps(du, C, Bvec, A, W):
    du = np.asarray(du, dtype=np.float32)
    C = np.asarray(C, dtype=np.float32)
    Bvec = np.asarray(Bvec, dtype=np.float32)
    A = np.asarray(A, dtype=np.float32)
    W = np.asarray(W, dtype=np.float32)

    # [b, p, ht, c, j] bf16, contiguous per core
    du_pre = np.ascontiguousarray(
        du.reshape(B, HC, P, NH, LH).transpose(0, 2, 3, 1, 4).astype(BF16_NP)
    )
    wt = W.T.reshape(HC, P, N).transpose(1, 0, 2).astype(BF16_NP)  # (P, HC, N)
    wt = np.ascontiguousarray(wt)
    ccbt = np.ascontiguousarray((C * Bvec[None, :]).T.astype(BF16_NP))  # (N, H)
    base = C @ (A @ Bvec)                                 # (H,)
    base_t = np.ascontiguousarray(base.reshape(HC, P).T)  # (P, HC)

    return [
        {"du": du_pre[b], "wt": wt, "ccbt": ccbt, "base": base_t}
        for b in range(B)
    ]


def run(du, C, Bvec, A, W, trace=False):
    nc = _get_nc()
    in_maps = _prep_in_maps(du, C, Bvec, A, W)
    res = run_bass_kernel_spmd(nc, in_maps, core_ids=list(range(B)), trace=trace)
    out_pre = np.stack([res.results[b]["out"] for b in range(B)], axis=0)
    # [b, p, lt, c, j] -> [b, c, p, lt, j] -> (B, H, L) f32
    out = (
        out_pre.transpose(0, 3, 1, 2, 4).astype(np.float32).reshape(B, H, L)
    )
    return out, res


def kernel(du, C, Bvec, A, W):
    out, _ = run(du, C, Bvec, A, W, trace=False)
    return out
